# revision 7
# baseline (speedup 1.0000x reference)
"""Trainium2 Bass kernel for CantorGlobalAttention (v3).

Math (per direction d, row r=(e,b), patch p), l_w = q*s_w:
  th  = tanh(q * d01h)              2-way blend (overflow-free)
  T5  = (cp5 + hn5*th)              = (v2 - A)/5
  z   = min(1 + e^{q*d02} + e^{q*d12}, C);  r = 1/z  (= s2)
  out_d/5 = v25 - T5 + T5*r
  OUT = c2s - sum_d T5_d + sum_d T5_d*r_d    (c2s = sum_d v2_d/5)

Engine layout per [128,2048] direction-phase tile:
  ACT: tanh, exp, exp (one table set - no table switches)
  GP : s = ea + eb (f32 out), T5 = th*hn5 + cp5 (bf16)
  DVE: z = min(s+1, C) (f32 2x), r = recip_approx(z), y = T5*r
  PE : acc += -I*T5 + I*y   (both direction-sum streams in PSUM),
       K/V row-sums via replicated-ones matmuls on transposed upload
       (M=128 fast path) + k=1 transpose matvecs + gather preludes.

All Q/K/V traffic is bf16 (tolerance 2e-2); output bf16, host casts.
Sharding: data-parallel over batch (dim 2), 8 cores x 8 batches.
"""

import numpy as np
import ml_dtypes
from contextlib import ExitStack

import concourse.bass as bass
import concourse.bacc as bacc
import concourse.tile as tile
from concourse import mybir
from concourse import bass_utils

F32 = mybir.dt.float32
BF16 = mybir.dt.bfloat16
AF = mybir.ActivationFunctionType
OP = mybir.AluOpType

D, E, B, P = 5, 16, 64, 4096
W = 3
NCORES = 8
BC = B // NCORES          # 8 batches per core
R = E * BC                # 128 rows = partitions, r = e*BC + b
EXPERT_DIM = 128

PH = 2048                 # phase width (cols per phase), P = 2*PH
NKV = 16                  # kv chunk-pairs per direction (32 blocks of 128)
MMF = 512                 # matmul max free dim (one PSUM bank)
CLAMP = 1e37


def _build_bass():
    nc = bacc.Bacc("TRN2", debug=False, num_devices=NCORES)
    q = nc.dram_tensor("q", [D, 2, R, PH], BF16, kind="ExternalInput").ap()
    # kvt[d, i, p, j, 0:128]=K^T block (2i+j), [128:256]=V^T block (2i+j)
    kvt = nc.dram_tensor("kvt", [D, NKV, 128, 2, 256], BF16,
                         kind="ExternalInput").ap()
    # 9 matrices: d01hT d02T d12T hn5T cp5T v25T -I +I ones
    mats = nc.dram_tensor("mats", [9, R, R], BF16, kind="ExternalInput").ap()
    out = nc.dram_tensor("out", [2, R, PH], BF16, kind="ExternalOutput").ap()

    with ExitStack() as ctx:
        tc = ctx.enter_context(tile.TileContext(nc))
        _body(ctx, tc, q, kvt, mats, out)
    if not nc.is_finalized():
        nc.finalize()
    return nc


def _body(ctx, tc, q, kvt, mats, out):
    nc = tc.nc
    singles = ctx.enter_context(tc.tile_pool(name="singles", bufs=1))

    qpool = ctx.enter_context(tc.tile_pool(name="qp", bufs=3))
    kv_pool = ctx.enter_context(tc.tile_pool(name="kv", bufs=8))
    wpool = ctx.enter_context(tc.tile_pool(name="wp", bufs=3))
    fpool = ctx.enter_context(tc.tile_pool(name="fp", bufs=2))
    opool = ctx.enter_context(tc.tile_pool(name="op", bufs=2))
    accp = ctx.enter_context(tc.tile_pool(name="accp", bufs=1, space="PSUM"))
    redp = ctx.enter_context(tc.tile_pool(name="redp", bufs=2, space="PSUM"))
    tpp = ctx.enter_context(tc.tile_pool(name="tpp", bufs=1, space="PSUM"))
    ppp = ctx.enter_context(tc.tile_pool(name="ppp", bufs=1, space="PSUM"))

    # --- constants ---
    mat_sb = []
    for i in range(9):
        m = singles.tile([R, R], BF16, tag=f"mat{i}")
        nc.sync.dma_start(out=m, in_=mats[i, :, :])
        mat_sb.append(m)
    (m_d01h, m_d02, m_d12, m_hn5, m_cp5, m_v25,
     m_negI, m_posI, m_ones) = mat_sb
    one_f32 = singles.tile([1, 1], F32, tag="one32")
    nc.vector.memset(one_f32, 1.0)

    consts = [singles.tile([R, 6], F32, tag=f"c{d}", name=f"c{d}")
              for d in range(D)]
    kvc = [singles.tile([R, 2], BF16, tag=f"kvc{d}", name=f"kvc{d}")
           for d in range(D)]
    c2s = singles.tile([R, 1], F32, tag="c2s")

    def emit_reduce(d):
        # K/V row-sums: replicated colsum (M=128 fast path) into psum
        red = redp.tile([128, 512], F32, tag="red")
        for i in range(NKV):
            t = kv_pool.tile([128, 2, 256], BF16, tag="kvt")
            nc.sync.dma_start(out=t, in_=kvt[d, i, :, :, :])
            nc.tensor.matmul(red, m_ones, t,
                             start=(i == 0), stop=(i == NKV - 1))
        row = wpool.tile([1, 512], F32, tag="row")
        nc.vector.tensor_copy(row, red[0:1, :])
        # fold even/odd halves while transposing row->column (k=1 mm)
        tp = tpp.tile([R, 2], F32, tag="tp")
        nc.tensor.matmul(tp[:, 0:1], row[0:1, 0:128], one_f32,
                         start=True, stop=False)
        nc.tensor.matmul(tp[:, 0:1], row[0:1, 256:384], one_f32,
                         start=False, stop=True)
        nc.tensor.matmul(tp[:, 1:2], row[0:1, 128:256], one_f32,
                         start=True, stop=False)
        nc.tensor.matmul(tp[:, 1:2], row[0:1, 384:512], one_f32,
                         start=False, stop=True)
        nc.vector.tensor_copy(kvc[d], tp)     # f32 psum -> bf16 cols
        kc, vc = kvc[d][:, 0:1], kvc[d][:, 1:2]
        pp = ppp.tile([R, 6], F32, tag="pp")
        for j, (mm, col) in enumerate(((m_d01h, kc), (m_d02, kc),
                                       (m_d12, kc), (m_hn5, vc),
                                       (m_cp5, vc), (m_v25, vc))):
            nc.tensor.matmul(pp[:, j:j + 1], mm, col, start=True, stop=True)
        nc.vector.tensor_copy(consts[d], pp)

    def emit_qphase(d, ph, acc):
        cd = consts[d]
        qt = qpool.tile([R, PH], BF16, tag="q")
        nc.sync.dma_start(out=qt, in_=q[d, ph, :, :])
        th = wpool.tile([R, PH], BF16, tag="th")
        nc.scalar.activation(out=th, in_=qt, func=AF.Tanh, scale=cd[:, 0:1])
        ea = wpool.tile([R, PH], BF16, tag="ea")
        nc.scalar.activation(out=ea, in_=qt, func=AF.Exp, scale=cd[:, 1:2])
        eb = wpool.tile([R, PH], BF16, tag="eb")
        nc.scalar.activation(out=eb, in_=qt, func=AF.Exp, scale=cd[:, 2:3])
        t5 = wpool.tile([R, PH], BF16, tag="t5")
        nc.gpsimd.tensor_scalar(out=t5, in0=th, scalar1=cd[:, 3:4],
                                scalar2=cd[:, 4:5], op0=OP.mult, op1=OP.add)
        s = fpool.tile([R, PH], F32, tag="s")
        nc.gpsimd.tensor_tensor(s, ea, eb, OP.add)
        z = fpool.tile([R, PH], F32, tag="z")
        nc.vector.tensor_scalar(out=z, in0=s, scalar1=1.0, scalar2=CLAMP,
                                op0=OP.add, op1=OP.min)
        r = fpool.tile([R, PH], F32, tag="r")
        nc.vector.reciprocal_approx_fast(out=r, in_=z)
        y = wpool.tile([R, PH], BF16, tag="y")
        nc.vector.tensor_tensor(y, t5, r, OP.mult)
        for pc in range(PH // MMF):
            sl = slice(pc * MMF, (pc + 1) * MMF)
            nc.tensor.matmul(acc[:, sl], m_negI, t5[:, sl],
                             start=(d == 0), stop=False)
            nc.tensor.matmul(acc[:, sl], m_posI, y[:, sl],
                             start=False, stop=(d == D - 1))
        return qt

    def emit_phase_out(ph, acc):
        osb = opool.tile([R, PH], BF16, tag="osb")
        nc.vector.tensor_scalar(out=osb, in0=acc, scalar1=1.0,
                                scalar2=c2s[:, 0:1], op0=OP.mult, op1=OP.add)
        nc.scalar.dma_start(out=out[ph, :, :], in_=osb)

    # --- schedule: reduces staggered two directions ahead of phase A ---
    emit_reduce(0)
    emit_reduce(1)
    accA = accp.tile([R, PH], F32, tag="acc", name="accA")
    for d in range(D):
        emit_qphase(d, 0, accA)
        if d + 2 < D:
            emit_reduce(d + 2)
        if d == D - 1:
            nc.vector.tensor_add(c2s, consts[0][:, 5:6], consts[1][:, 5:6])
            nc.vector.tensor_add(c2s, c2s, consts[2][:, 5:6])
            nc.vector.tensor_add(c2s, c2s, consts[3][:, 5:6])
            nc.vector.tensor_add(c2s, c2s, consts[4][:, 5:6])
    emit_phase_out(0, accA)
    accB = accp.tile([R, PH], F32, tag="acc", name="accB")
    for d in range(D):
        emit_qphase(d, 1, accB)
    emit_phase_out(1, accB)


def _host_constants(betas, temperature, routes):
    betas = np.asarray(betas, dtype=np.float32)
    routes = np.asarray(routes).astype(np.int64)
    temp = np.abs(np.asarray(temperature, dtype=np.float32).reshape(-1)[0])
    scale = np.float32(1.0) / (np.sqrt(np.float32(EXPERT_DIM)) * temp)

    self_idx = np.arange(E)
    gate = np.where(
        routes == self_idx[:, None], np.float32(1.0),
        (np.float32(1.0) / (np.float32(1.0) +
                            np.exp(-betas[self_idx[:, None], routes]))),
    ).astype(np.float32)  # [E, W]

    A = np.zeros((W, R, R), dtype=np.float32)   # s_w gather (scale*beta)
    G = np.zeros((W, R, R), dtype=np.float32)   # v_w gather (1/P folded)
    rows = np.arange(R)
    e_of_r = rows // BC
    b_of_r = rows % BC
    for w in range(W):
        cols = routes[e_of_r, w] * BC + b_of_r
        A[w, rows, cols] += scale * gate[e_of_r, w]
        G[w, rows, cols] += np.float32(1.0 / P)

    m_d01h = 0.5 * (A[0] - A[1])
    m_d02 = A[0] - A[2]
    m_d12 = A[1] - A[2]
    m_h = 0.5 * (G[0] - G[1])
    m_hn5 = -m_h / D
    m_cp5 = ((G[2] - G[1]) - m_h) / D
    m_v25 = G[2] / D
    negI = -np.eye(R, dtype=np.float32)
    posI = np.eye(R, dtype=np.float32)
    ones = np.ones((R, R), dtype=np.float32)
    mats = np.stack([m_d01h.T, m_d02.T, m_d12.T, m_hn5.T, m_cp5.T, m_v25.T,
                     negI, posI, ones])
    return np.ascontiguousarray(mats).astype(ml_dtypes.bfloat16)


_CACHE = {}


def kernel(Q, K, V, betas, temperature, routes, num_patches):
    Q = np.asarray(Q, dtype=np.float32)
    K = np.asarray(K, dtype=np.float32)
    V = np.asarray(V, dtype=np.float32)
    mats = _host_constants(betas, temperature, routes)

    if "nc" not in _CACHE:
        _CACHE["nc"] = _build_bass()
    nc = _CACHE["nc"]

    in_maps = []
    for i in range(NCORES):
        sl = slice(i * BC, (i + 1) * BC)
        Qc = Q[:, :, sl, :].reshape(D, R, P)
        Kc = K[:, :, sl, :].reshape(D, R, P)
        Vc = V[:, :, sl, :].reshape(D, R, P)
        qh = np.ascontiguousarray(
            Qc.reshape(D, R, 2, PH).transpose(0, 2, 1, 3)
        ).astype(ml_dtypes.bfloat16)
        # K^T/V^T blocks: [D, 32, 128, 128] -> packed [D, 16, 128, 2, 256]
        Kt = Kc.transpose(0, 2, 1).reshape(D, 32, 128, 128)
        Vt = Vc.transpose(0, 2, 1).reshape(D, 32, 128, 128)
        kvb = np.concatenate([Kt, Vt], axis=-1)          # [D, 32, 128, 256]
        kvh = np.ascontiguousarray(
            kvb.reshape(D, NKV, 2, 128, 256).transpose(0, 1, 3, 2, 4)
        ).astype(ml_dtypes.bfloat16)
        in_maps.append({"q": qh, "kvt": kvh, "mats": mats})

    res = bass_utils.run_bass_kernel_spmd(nc, in_maps,
                                          core_ids=list(range(NCORES)))
    _CACHE["last"] = res
    full = np.empty((B, E * P), dtype=np.float32)
    for i in range(NCORES):
        o = res.results[i]["out"].astype(np.float32)     # [2, R, PH]
        full[i * BC:(i + 1) * BC] = (
            o.reshape(2, E, BC, PH).transpose(2, 1, 0, 3).reshape(BC, E * P))
    return full


# revision 8
# speedup vs baseline: 1.1106x; 1.1106x over previous
"""Trainium2 Bass kernel for CantorGlobalAttention (v3).

Math (per direction d, row r=(e,b), patch p), l_w = q*s_w:
  th  = tanh(q * d01h)              2-way blend (overflow-free)
  T5  = (cp5 + hn5*th)              = (v2 - A)/5
  z   = min(1 + e^{q*d02} + e^{q*d12}, C);  r = 1/z  (= s2)
  out_d/5 = v25 - T5 + T5*r
  OUT = c2s - sum_d T5_d + sum_d T5_d*r_d    (c2s = sum_d v2_d/5)

Engine layout per [128,2048] direction-phase tile:
  ACT: tanh, exp, exp (one table set - no table switches)
  GP : s = ea + eb (f32 out), T5 = th*hn5 + cp5 (bf16)
  DVE: z = min(s+1, C) (f32 2x), r = recip_approx(z), y = T5*r
  PE : acc += -I*T5 + I*y   (both direction-sum streams in PSUM),
       K/V row-sums via replicated-ones matmuls on transposed upload
       (M=128 fast path) + k=1 transpose matvecs + gather preludes.

All Q/K/V traffic is bf16 (tolerance 2e-2); output bf16, host casts.
Sharding: data-parallel over batch (dim 2), 8 cores x 8 batches.
"""

import numpy as np
import ml_dtypes
from contextlib import ExitStack

import concourse.bass as bass
import concourse.bacc as bacc
import concourse.tile as tile
from concourse import mybir
from concourse import bass_utils

F32 = mybir.dt.float32
BF16 = mybir.dt.bfloat16
AF = mybir.ActivationFunctionType
OP = mybir.AluOpType

D, E, B, P = 5, 16, 64, 4096
W = 3
NCORES = 8
BC = B // NCORES          # 8 batches per core
R = E * BC                # 128 rows = partitions, r = e*BC + b
EXPERT_DIM = 128

PH = 2048                 # phase width (cols per phase), P = 2*PH
NKV = 16                  # kv chunk-pairs per direction (32 blocks of 128)
MMF = 512                 # matmul max free dim (one PSUM bank)
CLAMP = 1e37


def _build_bass():
    nc = bacc.Bacc("TRN2", debug=False, num_devices=NCORES)
    q = nc.dram_tensor("q", [D, 2, R, PH], BF16, kind="ExternalInput").ap()
    # kvt[d, i, p, j, 0:128]=K^T block (2i+j), [128:256]=V^T block (2i+j)
    kvt = nc.dram_tensor("kvt", [D, NKV, 128, 2, 256], BF16,
                         kind="ExternalInput").ap()
    # 9 matrices packed partition-major: mats[p, i, :] = M_i[p, :]
    mats = nc.dram_tensor("mats", [R, 9, R], BF16, kind="ExternalInput").ap()
    out = nc.dram_tensor("out", [2, R, PH], BF16, kind="ExternalOutput").ap()

    with ExitStack() as ctx:
        tc = ctx.enter_context(tile.TileContext(nc))
        _body(ctx, tc, q, kvt, mats, out)
    if not nc.is_finalized():
        nc.finalize()
    return nc


def _body(ctx, tc, q, kvt, mats, out):
    nc = tc.nc
    singles = ctx.enter_context(tc.tile_pool(name="singles", bufs=1))

    qpool = ctx.enter_context(tc.tile_pool(name="qp", bufs=3))
    kv_pool = ctx.enter_context(tc.tile_pool(name="kv", bufs=2))
    wpool = ctx.enter_context(tc.tile_pool(name="wp", bufs=3))
    fpool = ctx.enter_context(tc.tile_pool(name="fp", bufs=2))
    opool = ctx.enter_context(tc.tile_pool(name="op", bufs=2))
    accp = ctx.enter_context(tc.tile_pool(name="accp", bufs=1, space="PSUM"))
    redp = ctx.enter_context(tc.tile_pool(name="redp", bufs=2, space="PSUM"))
    tpp = ctx.enter_context(tc.tile_pool(name="tpp", bufs=1, space="PSUM"))
    ppp = ctx.enter_context(tc.tile_pool(name="ppp", bufs=1, space="PSUM"))

    # --- constants ---
    matst = singles.tile([R, 9, R], BF16, tag="mats")
    nc.sync.dma_start(out=matst, in_=mats)
    (m_d01h, m_d02, m_d12, m_hn5, m_cp5, m_v25,
     m_negI, m_posI, m_ones) = [matst[:, i, :] for i in range(9)]
    one_f32 = singles.tile([1, 1], F32, tag="one32")
    nc.vector.memset(one_f32, 1.0)

    consts = [singles.tile([R, 6], F32, tag=f"c{d}", name=f"c{d}")
              for d in range(D)]
    kvc = [singles.tile([R, 2], BF16, tag=f"kvc{d}", name=f"kvc{d}")
           for d in range(D)]
    c2s = singles.tile([R, 1], F32, tag="c2s")

    def emit_reduce(d):
        # K/V row-sums: replicated colsum (M=128 fast path) into psum
        red = redp.tile([128, 512], F32, tag="red")
        t = kv_pool.tile([128, NKV, 2, 256], BF16, tag="kvt")
        nc.sync.dma_start(out=t, in_=kvt[d, :, :, :, :])
        for i in range(NKV):
            nc.tensor.matmul(red, m_ones, t[:, i, :, :],
                             start=(i == 0), stop=(i == NKV - 1))
        row = wpool.tile([1, 512], F32, tag="row")
        nc.vector.tensor_copy(row, red[0:1, :])
        # fold even/odd halves while transposing row->column (k=1 mm)
        tp = tpp.tile([R, 2], F32, tag="tp")
        nc.tensor.matmul(tp[:, 0:1], row[0:1, 0:128], one_f32,
                         start=True, stop=False)
        nc.tensor.matmul(tp[:, 0:1], row[0:1, 256:384], one_f32,
                         start=False, stop=True)
        nc.tensor.matmul(tp[:, 1:2], row[0:1, 128:256], one_f32,
                         start=True, stop=False)
        nc.tensor.matmul(tp[:, 1:2], row[0:1, 384:512], one_f32,
                         start=False, stop=True)
        nc.vector.tensor_copy(kvc[d], tp)     # f32 psum -> bf16 cols
        kc, vc = kvc[d][:, 0:1], kvc[d][:, 1:2]
        pp = ppp.tile([R, 6], F32, tag="pp")
        for j, (mm, col) in enumerate(((m_d01h, kc), (m_d02, kc),
                                       (m_d12, kc), (m_hn5, vc),
                                       (m_cp5, vc), (m_v25, vc))):
            nc.tensor.matmul(pp[:, j:j + 1], mm, col, start=True, stop=True)
        nc.vector.tensor_copy(consts[d], pp)

    def emit_qphase(d, ph, acc):
        cd = consts[d]
        qt = qpool.tile([R, PH], BF16, tag="q")
        nc.sync.dma_start(out=qt, in_=q[d, ph, :, :])
        th = wpool.tile([R, PH], BF16, tag="th")
        nc.scalar.activation(out=th, in_=qt, func=AF.Tanh, scale=cd[:, 0:1])
        ea = wpool.tile([R, PH], BF16, tag="ea")
        nc.scalar.activation(out=ea, in_=qt, func=AF.Exp, scale=cd[:, 1:2])
        eb = wpool.tile([R, PH], BF16, tag="eb")
        nc.scalar.activation(out=eb, in_=qt, func=AF.Exp, scale=cd[:, 2:3])
        t5 = wpool.tile([R, PH], BF16, tag="t5")
        t5_eng = nc.gpsimd if (2 * d + ph) % 2 == 0 else nc.vector
        t5_eng.tensor_scalar(out=t5, in0=th, scalar1=cd[:, 3:4],
                             scalar2=cd[:, 4:5], op0=OP.mult, op1=OP.add)
        s = fpool.tile([R, PH], F32, tag="s")
        nc.gpsimd.tensor_tensor(s, ea, eb, OP.add)
        z = fpool.tile([R, PH], F32, tag="z")
        nc.vector.tensor_scalar(out=z, in0=s, scalar1=1.0, scalar2=CLAMP,
                                op0=OP.add, op1=OP.min)
        r = fpool.tile([R, PH], F32, tag="r")
        nc.vector.reciprocal_approx_fast(out=r, in_=z)
        u16 = wpool.tile([R, PH], BF16, tag="u16")
        nc.vector.tensor_scalar(out=u16, in0=r, scalar1=-1.0, scalar2=1.0,
                                op0=OP.mult, op1=OP.add)   # u = 1-s2, bf16
        y = wpool.tile([R, PH], BF16, tag="y")
        nc.vector.tensor_tensor(y, t5, u16, OP.mult)       # T5*(1-s2)
        for pc in range(PH // MMF):
            sl = slice(pc * MMF, (pc + 1) * MMF)
            nc.tensor.matmul(acc[:, sl], m_negI, y[:, sl],
                             start=(d == 0), stop=(d == D - 1))
        return qt

    def emit_phase_out(ph, acc):
        osb = opool.tile([R, PH], BF16, tag="osb")
        nc.vector.tensor_scalar(out=osb, in0=acc, scalar1=1.0,
                                scalar2=c2s[:, 0:1], op0=OP.mult, op1=OP.add)
        nc.scalar.dma_start(out=out[ph, :, :], in_=osb)

    # --- schedule: reduces staggered two directions ahead of phase A ---
    emit_reduce(0)
    emit_reduce(1)
    accA = accp.tile([R, PH], F32, tag="acc", name="accA")
    for d in range(D):
        emit_qphase(d, 0, accA)
        if d + 2 < D:
            emit_reduce(d + 2)
        if d == D - 1:
            nc.vector.tensor_add(c2s, consts[0][:, 5:6], consts[1][:, 5:6])
            nc.vector.tensor_add(c2s, c2s, consts[2][:, 5:6])
            nc.vector.tensor_add(c2s, c2s, consts[3][:, 5:6])
            nc.vector.tensor_add(c2s, c2s, consts[4][:, 5:6])
    emit_phase_out(0, accA)
    accB = accp.tile([R, PH], F32, tag="acc", name="accB")
    for d in range(D):
        emit_qphase(d, 1, accB)
    emit_phase_out(1, accB)


def _host_constants(betas, temperature, routes):
    betas = np.asarray(betas, dtype=np.float32)
    routes = np.asarray(routes).astype(np.int64)
    temp = np.abs(np.asarray(temperature, dtype=np.float32).reshape(-1)[0])
    scale = np.float32(1.0) / (np.sqrt(np.float32(EXPERT_DIM)) * temp)

    self_idx = np.arange(E)
    gate = np.where(
        routes == self_idx[:, None], np.float32(1.0),
        (np.float32(1.0) / (np.float32(1.0) +
                            np.exp(-betas[self_idx[:, None], routes]))),
    ).astype(np.float32)  # [E, W]

    A = np.zeros((W, R, R), dtype=np.float32)   # s_w gather (scale*beta)
    G = np.zeros((W, R, R), dtype=np.float32)   # v_w gather (1/P folded)
    rows = np.arange(R)
    e_of_r = rows // BC
    b_of_r = rows % BC
    for w in range(W):
        cols = routes[e_of_r, w] * BC + b_of_r
        A[w, rows, cols] += scale * gate[e_of_r, w]
        G[w, rows, cols] += np.float32(1.0 / P)

    m_d01h = 0.5 * (A[0] - A[1])
    m_d02 = A[0] - A[2]
    m_d12 = A[1] - A[2]
    m_h = 0.5 * (G[0] - G[1])
    m_hn5 = -m_h / D
    m_cp5 = ((G[2] - G[1]) - m_h) / D
    m_v25 = G[2] / D
    negI = -np.eye(R, dtype=np.float32)
    posI = np.eye(R, dtype=np.float32)
    ones = np.ones((R, R), dtype=np.float32)
    mats = np.stack([m_d01h.T, m_d02.T, m_d12.T, m_hn5.T, m_cp5.T, m_v25.T,
                     negI, posI, ones])                   # [9, R, R]
    mats = mats.transpose(1, 0, 2)                        # [R, 9, R]
    return np.ascontiguousarray(mats).astype(ml_dtypes.bfloat16)


_CACHE = {}


def kernel(Q, K, V, betas, temperature, routes, num_patches):
    Q = np.asarray(Q, dtype=np.float32)
    K = np.asarray(K, dtype=np.float32)
    V = np.asarray(V, dtype=np.float32)
    mats = _host_constants(betas, temperature, routes)

    if "nc" not in _CACHE:
        _CACHE["nc"] = _build_bass()
    nc = _CACHE["nc"]

    in_maps = []
    for i in range(NCORES):
        sl = slice(i * BC, (i + 1) * BC)
        Qc = Q[:, :, sl, :].reshape(D, R, P)
        Kc = K[:, :, sl, :].reshape(D, R, P)
        Vc = V[:, :, sl, :].reshape(D, R, P)
        qh = np.ascontiguousarray(
            Qc.reshape(D, R, 2, PH).transpose(0, 2, 1, 3)
        ).astype(ml_dtypes.bfloat16)
        # K^T/V^T blocks: [D, 32, 128, 128] -> packed [D, 16, 128, 2, 256]
        Kt = Kc.transpose(0, 2, 1).reshape(D, 32, 128, 128)
        Vt = Vc.transpose(0, 2, 1).reshape(D, 32, 128, 128)
        kvb = np.concatenate([Kt, Vt], axis=-1)          # [D, 32, 128, 256]
        kvh = np.ascontiguousarray(
            kvb.reshape(D, NKV, 2, 128, 256).transpose(0, 1, 3, 2, 4)
        ).astype(ml_dtypes.bfloat16)
        in_maps.append({"q": qh, "kvt": kvh, "mats": mats})

    res = bass_utils.run_bass_kernel_spmd(nc, in_maps,
                                          core_ids=list(range(NCORES)))
    _CACHE["last"] = res
    full = np.empty((B, E * P), dtype=np.float32)
    for i in range(NCORES):
        o = res.results[i]["out"].astype(np.float32)     # [2, R, PH]
        full[i * BC:(i + 1) * BC] = (
            o.reshape(2, E, BC, PH).transpose(2, 1, 0, 3).reshape(BC, E * P))
    return full
